# revision 2
# baseline (speedup 1.0000x reference)
"""Malvar demosaic on 8 trn2 NeuronCores — v3 (row-major bands).

Input CFA [16,1,1024,1024] f32 + four 5x5 kernels -> [16,3,1024,1024] f32.
Pure data parallel: 2 images/core, 9 bands of 124 output rows per image.

Measured-on-HW design points (axon pipelined-dispatch slope benches):
  - Stores are the wall: HBM-write DMAs cost ~360ns per 4KB descriptor
    per SDMA engine; both HWDGE rings (sync+scalar) share one ~127GB/s
    pool, the SWDGE (gpsimd) path adds capacity sublinearly. Best split
    found: ch0/ch2 stores on gpsimd, ch1 on sync (with loads on sync).
    Larger descriptors would fix this (12KB descs measured 640GB/s) but
    the channel-plane output layout caps SBUF-contiguity at one 4KB row
    per partition, and SBUF->SBUF fold DMAs measured 4-10x slower still.
  - HBM reads pipeline well (no receipt serialization): loads are cheap
    on any queue. Row-major xt layout (partition p = input row r0-2+p)
    makes each band ONE contiguous 512KB load; weight matrices absorb
    the layout (lhsT column d taps partitions d..d+4).
  - bf16 conv stack (lhsT weights exact in bf16; input rounding ~2^-9).
    Per (channel, col-parity): X/S1/S2 passes (base18), 18 matmuls per
    band accumulated in 3 two-bank PSUM tiles; per-channel evictions
    interleave both column parities in one strided copy.
"""

import numpy as np

import concourse.bass as bass
import concourse.mybir as mybir
import concourse.tile as tile
from concourse.bass_utils import run_bass_kernel_spmd

B, H, W = 16, 1024, 1024
N_CORES = 8
IMGS_PER_CORE = B // N_CORES
BAND = 124
NBANDS = (H + BAND - 1) // BAND
M = 124

# source per (channel, row-parity, col-parity): conv index 0..3 or "X"
_SEL = {
    (0, 0, 0): "X", (0, 0, 1): 1, (0, 1, 0): 2, (0, 1, 1): 3,   # R
    (1, 0, 0): 0, (1, 0, 1): "X", (1, 1, 0): "X", (1, 1, 1): 0,  # G
    (2, 0, 0): 3, (2, 0, 1): 2, (2, 1, 0): 1, (2, 1, 1): "X",    # B
}

LOAD_ENG = "sync"
STORE_PATTERN = ("gpsimd", "sync", "gpsimd")  # by (band_ctr*3 + ch) % len
CONV_ENG = "scalar"
EVICT_ENGS = ("vector", "scalar", "vector")
ZPAD_ENG = "gpsimd"
REPEAT = 1
HW_LOOP = 0     # if >0: wrap the repeat body in a hardware For_i loop
BUFS = 8        # x-tile buffers
SBUFS = None    # xb/s1/s2 buffers (default BUFS)
OBUFS = None    # plane buffers (default BUFS)


def _build_matrices(k5s):
    """Packed lhsT [128, 18*M] bf16; per (ch, cp): X / S1 / S2 planes.

    Row-major: xt partition p holds input row r0 - 2 + p; output row d
    (lhsT column d) needs partitions d .. d+4.
    """
    packed = np.zeros((128, 18 * M), dtype=np.float32)
    idx = 0
    for ch in range(3):
        for cp in range(2):
            Ms = [np.zeros((128, M), dtype=np.float32) for _ in range(3)]
            for d in range(BAND):
                src = _SEL[(ch, d % 2, cp)]
                if src == "X":
                    Ms[0][d + 2, d] += 1.0
                    continue
                k5 = k5s[src]
                for dy in range(-2, 3):
                    p = d + 2 + dy
                    Ms[0][p, d] += k5[2 + dy, 2]
                    Ms[1][p, d] += k5[2 + dy, 1]
                    Ms[2][p, d] += k5[2 + dy, 0]
            for pl in range(3):
                packed[:, idx * M:(idx + 1) * M] = Ms[pl]
                idx += 1
    import ml_dtypes
    return packed.astype(ml_dtypes.bfloat16)


def _split_waits(nc, max_waits=1):
    """Walrus rejects >1 sem wait per instruction; hoist extras onto NoOps."""
    total = 0
    for bb in nc.main_func.blocks:
        insts = bb.bb.instructions if hasattr(bb, "bb") else bb.instructions
        i = 0
        while i < len(insts):
            ins = insts[i]
            si = ins.sync_info
            if si is not None and si.on_wait and len(si.on_wait) > max_waits:
                waits = list(si.on_wait)
                keep, hoist = waits[-max_waits:], waits[:-max_waits]
                nops = []
                for w in hoist:
                    nop = mybir.InstNoOp(
                        name=nc.get_next_instruction_name(),
                        engine=ins.engine, ins=[], outs=[],
                        sync_info=mybir.SyncInfo(on_wait=[w], on_update=[]))
                    nc.register_instruction(nop)
                    nops.append(nop)
                ins.sync_info = mybir.SyncInfo(
                    on_wait=keep, on_update=list(si.on_update or []))
                insts[i:i] = nops
                i += len(nops)
                total += len(nops)
            i += 1
    return total


def _build_nc():
    bf16 = mybir.dt.bfloat16
    f32 = mybir.dt.float32
    nc = bass.Bass(target_bir_lowering=False, trn_type="TRN2")
    x = nc.dram_tensor("x", [IMGS_PER_CORE, 1, H, W], mybir.dt.float32r,
                       kind="ExternalInput")
    wts = nc.dram_tensor("wm", [128, 18 * M], bf16, kind="ExternalInput")
    zpad = nc.dram_tensor("zpad", [2, W], mybir.dt.float32r,
                          kind="ExternalInput")
    out = nc.dram_tensor("out", [IMGS_PER_CORE, 3, H, W], f32,
                         kind="ExternalOutput")

    def eng(name):
        return getattr(nc, name)

    sbufs = SBUFS if SBUFS is not None else BUFS
    obufs = OBUFS if OBUFS is not None else BUFS
    with tile.TileContext(nc) as tc:
        with (
            tc.tile_pool(name="wpool", bufs=1) as wpool,
            tc.tile_pool(name="xpool", bufs=BUFS) as xpool,
            tc.tile_pool(name="spool", bufs=sbufs) as spool,
            tc.tile_pool(name="opool", bufs=obufs) as opool,
            tc.tile_pool(name="psum", bufs=1, space="PSUM") as pspool,
        ):
            wt = wpool.tile([128, 18 * M], bf16)
            nc.gpsimd.dma_start(wt[:], wts[:])

            # pre-zero edge columns of every x buffer once (loads never
            # touch cols 0:2 / W+2:W+4)
            xts = [xpool.tile([128, W + 4], mybir.dt.float32r, tag="x",
                              name=f"xt{i}") for i in range(BUFS)]
            for xt_ in xts:
                nc.gpsimd.memset(xt_[:, 0:2].bitcast(f32), 0.0)
                nc.gpsimd.memset(xt_[:, W + 2:W + 4].bitcast(f32), 0.0)

            def band_iter(b, t, it):
                    r0 = t * BAND
                    n_rows = min(BAND, H - r0)
                    lo = max(r0 - 2, 0)           # first valid input row
                    hi = min(r0 + BAND + 2, H)    # one-past-last valid row
                    p_lo = lo - (r0 - 2)          # partition of row `lo`
                    p_hi = p_lo + (hi - lo)

                    xt = xts[it % BUFS]
                    eng(LOAD_ENG).dma_start(xt[p_lo:p_hi, 2:W + 2],
                                            x[b, 0, lo:hi, :])
                    if p_lo > 0:       # top edge: zero rows above image
                        eng(ZPAD_ENG).dma_start(xt[0:p_lo, 2:W + 2],
                                                zpad[0:p_lo, :])
                    if t == NBANDS - 1:  # bottom edge: rows H, H+1
                        eng(ZPAD_ENG).dma_start(xt[p_hi:p_hi + 2, 2:W + 2],
                                                zpad[:, :])

                    xb = spool.tile([128, W + 4], bf16, tag="xb")
                    if CONV_ENG == "scalar":
                        nc.scalar.copy(xb[:], xt[:])
                    else:
                        eng(CONV_ENG).tensor_copy(xb[:], xt[:])

                    s1 = spool.tile([128, W], bf16, tag="s1")
                    nc.vector.tensor_tensor(s1[:], xb[:, 1:W + 1],
                                            xb[:, 3:W + 3],
                                            mybir.AluOpType.add)
                    s2 = spool.tile([128, W], bf16, tag="s2")
                    nc.vector.tensor_tensor(s2[:], xb[:, 0:W],
                                            xb[:, 4:W + 4],
                                            mybir.AluOpType.add)

                    plane = opool.tile([128, 3 * W], f32, tag="pl")

                    for ch in range(3):
                        ps = pspool.tile([M, 2, 512], f32, tag=f"ps{ch}",
                                         name=f"ps{ch}")
                        for cp in range(2):
                            wbase = (ch * 2 + cp) * 3
                            passes = (
                                (wbase + 0, xb[:, 2 + cp:2 + cp + W:2]),
                                (wbase + 1, s1[:, cp:W:2]),
                                (wbase + 2, s2[:, cp:W:2]),
                            )
                            for pi, (wsl, rhs) in enumerate(passes):
                                nc.tensor.matmul(
                                    ps[:, cp, :],
                                    wt[:, wsl * M:(wsl + 1) * M], rhs,
                                    start=(pi == 0),
                                    stop=(pi == len(passes) - 1))
                        # interleave both parities in one eviction:
                        # dst cols ch*W + 2*j + cp  <-  ps[:, cp, j]
                        dst = plane[0:M, ch * W:(ch + 1) * W].rearrange(
                            "p (j c) -> p j c", c=2)
                        ee = EVICT_ENGS[ch]
                        src = ps[:, :, :].transpose([0, 2, 1])
                        if ee == "scalar":
                            nc.scalar.copy(dst, src)
                        else:
                            eng(ee).tensor_copy(dst, src)

                    for ch in range(3):
                        se = STORE_PATTERN[(it * 3 + ch) % len(STORE_PATTERN)]
                        eng(se).dma_start(
                            out[b, ch, r0:r0 + n_rows, :],
                            plane[0:n_rows, ch * W:(ch + 1) * W])

            def rep_body():
                it = 0
                for b in range(IMGS_PER_CORE):
                    for t in range(NBANDS):
                        band_iter(b, t, it)
                        it += 1

            if HW_LOOP > 0:
                with tc.For_i(0, HW_LOOP):
                    rep_body()
            else:
                for _rep in range(REPEAT):
                    rep_body()

    _split_waits(nc)
    nc.finalize()
    return nc


_CACHE = {}


def _get_nc():
    if "nc" not in _CACHE:
        _CACHE["nc"] = _build_nc()
    return _CACHE["nc"]


def kernel(CFA_inputs, GR_GB, Rg_RB_Bg_BR, Rg_BR_Bg_RB, Rb_BB_Br_RR,
           _trace=False):
    cfa = np.ascontiguousarray(np.asarray(CFA_inputs, dtype=np.float32))
    k5s = [np.asarray(k, dtype=np.float32)
           for k in (GR_GB, Rg_RB_Bg_BR, Rg_BR_Bg_RB, Rb_BB_Br_RR)]
    nc = _get_nc()

    wm = _build_matrices(k5s)
    zpad = np.zeros((2, W), dtype=np.float32)
    in_maps = [{"x": cfa[c * IMGS_PER_CORE:(c + 1) * IMGS_PER_CORE],
                "wm": wm, "zpad": zpad} for c in range(N_CORES)]

    res = run_bass_kernel_spmd(nc, in_maps, core_ids=list(range(N_CORES)),
                               trace=_trace)
    outs = np.concatenate([res.results[c]["out"] for c in range(N_CORES)],
                          axis=0)
    if _trace:
        kernel._last = res
    return outs
